# revision 17
# baseline (speedup 1.0000x reference)
"""Depthwise 4x4 FIR blur (upfirdn2d-style) on 8 Trainium2 NeuronCores.

Input  x: (16, 512, 64, 64) f32, kernel: (4, 4) f32 (normalized binomial).
Output y: same shape as x, y[g] = conv2d(zero-pad(x[g], (2,1)x(2,1)), flip(kernel)).

Equivalent per-image formula (derived from the reference):
    y[i, j] = sum_{a,b in [0,4)} kernel[a, b] * x[i+1-a, j+1-b]   (zero outside)

Strategy (per core, 1024 images = 16 strips of 64):
  - Host prepads each strip into [128, 2116]: partition k in [0,64) = row k of
    the even image of a pair, k in [64,128) = row k-64 of the odd image; along
    the free dim 32 image pairs at stride 66 (64 data cols + 2 zero cols) plus
    4 lead zeros. Horizontal taps then become free-dim shifts whose
    out-of-image reads land on zeros; strips load as one dense 1MB DMA.
  - The whole separable conv runs on the TensorEngine: for each horizontal tap
    b, a banded 128x128 matrix W_b (vertical taps folded in, block-diagonal
    per image) multiplies a shifted slice of the strip; 4 float32r matmuls
    accumulate per PSUM bank (1 cycle/row at N>=256).
  - ScalarE evacuates PSUM -> packed SBUF out tile and issues the stores
    (same engine -> the store needs no data semaphore), which go out in the
    natural [images, 64, 64] layout.
"""

import numpy as np

import concourse.bass as bass
import concourse.tile as tile
from concourse import mybir
from concourse.bass_utils import run_bass_kernel_spmd

# The kernel-tail drain waits on every semaphore family the kernel touched
# (PE + ACT + up to 8 DMA lanes); walrus rejects instructions with that many
# sync waits. Split the drain into several drain instructions, each carrying
# at most 3 waits — semantically identical (SP executes them in sequence).
import bass_rust as _bass_rust
from concourse.tile_scheduler import N_PROCS as _N_PROCS


def _split_drain_and_barrier(self, tick_clock, wait_clock):
    ScopedClock = _bass_rust.ScopedClock
    VectorClock = _bass_rust.VectorClock
    gc = tick_clock.global_clock
    vals = [gc[p] for p in range(_N_PROCS)]
    nonzero = [p for p in range(_N_PROCS) if vals[p] > 0]
    for p in nonzero:
        pv = [vals[q] if q == p else 0 for q in range(_N_PROCS)]
        d = self.nc.sync.drain()
        wait_clock.add_sem_waits(d.ins, ScopedClock({None: VectorClock(pv)}))
    self.nc.sync.drain()

    self.nc.all_engine_barrier()
    assert self.sems is not None
    popped = self.nc._tile_sem_poison_stack.pop()
    assert popped is self._sem_poison
    self.nc.clear_and_free_semaphores(list(self.sems.allocated().values()))
    self.nc.all_engine_barrier()


tile.TileContext._drain_and_barrier = _split_drain_and_barrier

# Partition HWDGE DMA-completion lanes by issuing engine: SP (loads) cycles
# lanes 0-5, ACT (stores) alternates lanes 6-7. A DMA must wait for the
# previous DMA on its lane (sem-value determinism); with dedicated store
# lanes that predecessor is store(s-2), whose completion the evacuation
# "poke" already made ACT observe — so the wait elides and every store keeps
# a single sem wait (walrus limit).
import concourse.tile_sem_assignment as _tsa
from concourse import bass_isa as _bass_isa


def _assign_tick_lane_split(self, inst):
    engine = inst.engine
    eng_proc_idx = (
        _tsa.ENGINE_SEQUENCER_TO_IDX if inst.is_sequencer_only() else _tsa.ENGINE_TO_IDX
    )[engine]
    if isinstance(inst, _tsa.DMAInst) and not isinstance(
        inst, _bass_isa.UserSyncedRemoteDMADescs
    ):
        if engine == mybir.EngineType.Pool:
            inst_proc_idx = _tsa.PROC_NAME_TO_IDX[f"DMASW{self.next_sw_dma_idx}"]
            self.next_sw_dma_idx = (self.next_sw_dma_idx + 1) % self.swdge_sem_count
        elif engine == mybir.EngineType.Activation:
            n = getattr(self, "_act_dma_count", 0)
            inst_proc_idx = _tsa.PROC_NAME_TO_IDX[f"DMAHW{6 + (n % 2)}"]
            self._act_dma_count = n + 1
        else:
            inst_proc_idx = _tsa.PROC_NAME_TO_IDX[f"DMAHW{self.next_hw_dma_idx}"]
            self.next_hw_dma_idx = (self.next_hw_dma_idx + 1) % 6
    elif isinstance(inst, mybir.InstCollectiveCompute):
        inst_proc_idx = _tsa.PROC_NAME_TO_IDX["Collectives"]
    else:
        inst_proc_idx = eng_proc_idx

    if not inst.is_executable():
        if not isinstance(inst, _tsa.BassTileCriticalSection):
            return
    if isinstance(inst, _bass_isa.InstPseudoReloadLibraryIndex):
        return

    if inst.descendants or isinstance(inst, _tsa._DMA_OR_COLLECTIVE_TYPES):
        inst.bass_scheduled_tick = self.global_clock.advance(inst_proc_idx)
        inst.bass_scheduled_proc = inst_proc_idx
        inst.bass_scheduled_scope = self.scope_name
        self._proc_insts[self.root_scope_name][inst_proc_idx].append(inst)
        if getattr(inst, "gen_mode", 0) == 1 and inst_proc_idx != eng_proc_idx:
            eng_tick = self.global_clock.advance(eng_proc_idx)
            self.tc.prep_eng_ticks[inst.name] = (eng_proc_idx, eng_tick)
            self._prep_eng_names[self.root_scope_name].append(inst.name)


_tsa.TileClockTick._assign_tick = _assign_tick_lane_split

N_CORES = 8
H = W = 64
SLOT = 66                       # free-dim stride per image (64 data + 2 zero)
LEAD = 4                        # leading zero cols in a strip
S = 32                          # image pairs (slots) per strip
STRIP_W = LEAD + SLOT * S       # 2116 f32 per partition
N_STRIPS = 16                   # strips per core (16 * 64 = 1024 images)
# chunk = slot range processed by one PSUM bank (<=512 f32 out cols)
CHUNKS = [(0, 7), (7, 14), (14, 21), (21, 28), (28, 32)]

F32 = mybir.dt.float32
F32R = mybir.dt.float32r


def _chunk_geom(t0, t1):
    ns = t1 - t0
    n_cols = SLOT * (ns - 1) + 64          # contiguous out span incl. gaps
    o = LEAD + SLOT * t0                   # first data col of the chunk
    return ns, n_cols, o


def build_nc(n_strips: int, mm_dtype=F32R, relax: bool = True):
    """Build the Bass program for one core processing n_strips*64 images.

    Sync-topology note: walrus allows only ONE semaphore wait on most
    instruction structs (matmul/ldweights, DMA pseudo), so the program is
    shaped so every instruction has at most one cross-engine dependency:
      - each strip gets its own SBUF x tile -> loads have NO deps at all
        (pure prefetch, all queued on the SP HWDGE ring up front);
      - tiny "absorber" matmuls fold the load-DMA wait and the PSUM-WAR
        (previous strip's DVE evacuations) into PE program order;
      - a 1-element DVE poke folds the out-buffer WAR (store of strip s-2)
        into DVE program order before the real evacuations.
    """
    from concourse.tile_rust import add_dep_helper as _adh
    from concourse.tile_scheduler import DMAInst

    def add_dep_helper(a, b, sync=False, reason=""):
        _adh(getattr(a, "ins", a), getattr(b, "ins", b), sync=sync, reason=reason)

    def relax_same_engine_deps(nc):
        """Demote same-engine compute->compute sync deps to order-only.

        Engines execute and complete their compute queues strictly in order,
        so a same-engine dependency never needs a semaphore — but Tile emits
        one anyway (self-waits), and walrus allows only a single sem wait on
        most instruction structs. DMA producers/consumers are excluded: a DMA
        instruction's completion is asynchronous to its issuing engine.
        """
        imap = nc.inst_map
        for inst in nc.all_instructions():
            if isinstance(inst, DMAInst) or not inst.is_executable():
                continue
            if inst.is_sequencer_only():
                continue
            sync_names = list(inst.sync_dependency_names())
            move = []
            for dn in sync_names:
                prod = imap.get(dn)
                if prod is None or isinstance(prod, DMAInst):
                    continue
                if not prod.is_executable() or prod.is_sequencer_only():
                    continue
                if prod.engine == inst.engine:
                    move.append(dn)
            if move:
                sync_set = inst.sync_dependency_set_copy()
                nosync_set = inst.nosync_dependency_set_copy()
                for dn in move:
                    sync_set.discard(dn)
                    nosync_set.add(dn)
                inst.set_sync_dependencies(sync_set)
                inst.set_nosync_dependencies(nosync_set)

    n_images = n_strips * 2 * S
    nc = bass.Bass(
        "TRN2", target_bir_lowering=False, detect_race_conditions=not relax
    )
    x_dram = nc.dram_tensor(
        "x", [n_strips, 128, STRIP_W], mm_dtype, kind="ExternalInput"
    )
    w_dram = nc.dram_tensor("w", [128, 512], mm_dtype, kind="ExternalInput")
    y_dram = nc.dram_tensor("y", [n_images, H, W], F32, kind="ExternalOutput")

    with tile.TileContext(nc) as tc:
        with (
            tc.tile_pool(name="pers", bufs=1) as pers,
            tc.tile_pool(name="psum", bufs=7, space="PSUM") as pp,
        ):
            wt = pers.tile([128, 512], mm_dtype, tag="wt")
            nc.sync.dma_start(wt[:], w_dram[:])

            x_tiles = [
                pers.tile([128, STRIP_W], mm_dtype, tag=f"xs{i}", name=f"xst{i}")
                for i in range(n_strips)
            ]
            y_bufs = [
                pers.tile([128, 64 * S], F32, tag=f"y{i}", name=f"ybuf{i}")
                for i in range(2)
            ]

            # prefetch every strip: no deps -> no waits, SP ring streams them
            for s in range(n_strips):
                nc.sync.dma_start(x_tiles[s][:], x_dram[s])

            # scratch PSUM tile for the absorber matmuls
            warm = pp.tile([128, 128], F32, name="warm", tag="warm", bufs=1)
            prev_mm = nc.tensor.matmul(
                warm[:], wt[:, 0:128], wt[:, 0:128], start=True, stop=True
            )

            for s in range(n_strips):
                xb = x_tiles[s]
                yb = y_bufs[s % 2]
                yb_prev = y_bufs[(s - 1) % 2]

                # absorber 1: load(s) completion -> PE program order
                d1 = nc.tensor.matmul(
                    warm[:, 0:4], wt[:, 0:128], xb[:, 0:4], start=True, stop=True
                )
                add_dep_helper(d1, prev_mm, sync=False, reason="strip order")
                gate = d1
                if s >= 1:
                    # absorber 2: strip s-1 PSUM evacuations (DVE) -> PE order.
                    # Reads the last column block chunk-4's copy wrote, so its
                    # single wait covers every previous evacuation tick.
                    pk = yb_prev[:, 64 * S - 4 : 64 * S]
                    d2 = nc.tensor.matmul(
                        warm[0:4, 4:8], pk, pk, start=True, stop=True
                    )
                    add_dep_helper(d2, d1, sync=False, reason="absorber order")
                    gate = d2

                # ---- 4 banded matmuls per chunk, accumulated in PSUM ----
                psum_tiles = [
                    pp.tile([128, 512], F32, name=f"ps{s}_{ci}", tag="ps")
                    for ci in range(len(CHUNKS))
                ]
                first_mms = []
                for b in range(4):
                    lhsT = wt[:, 128 * b : 128 * (b + 1)]
                    d = 1 - b                      # horizontal tap shift
                    for ci, (t0, t1) in enumerate(CHUNKS):
                        ns, n_cols, o = _chunk_geom(t0, t1)
                        rhs = xb[:, o + d : o + d + n_cols]
                        mm = nc.tensor.matmul(
                            psum_tiles[ci][:, 0:n_cols],
                            lhsT,
                            rhs,
                            start=(b == 0),
                            stop=(b == 3),
                        )
                        if b == 0:
                            add_dep_helper(mm, gate, sync=False, reason="gate")
                        prev_mm = mm

                # absorber 3: store(s-2) completion -> ACT program order.
                # Pokes one element of yb (chunk-0's copy rewrites it next).
                d3 = nc.scalar.copy(yb[0:1, 0:1], wt[0:1, 0:1].bitcast(F32))

                # ---- evacuate PSUM -> packed out tile (VectorE) ----
                copies = []
                for ci, (t0, t1) in enumerate(CHUNKS):
                    ns, n_cols, o = _chunk_geom(t0, t1)
                    src_c = psum_tiles[ci][:, 0 : SLOT * ns].rearrange(
                        "p (t u) -> p t u", u=SLOT
                    )[:, :, 0:64]
                    dst_c = yb[:, 64 * t0 : 64 * t1].rearrange(
                        "p (t w) -> p t w", w=64
                    )
                    cp = nc.scalar.copy(dst_c, src_c)
                    add_dep_helper(cp, d3, sync=False, reason="poke order")
                    copies.append(cp)

                # ---- store to the natural [images, 64, 64] layout ----
                st_dst = y_dram[64 * s : 64 * s + 64].rearrange(
                    "(t j) r w -> (j r) t w", t=S, j=2
                )
                st_src = yb[:].rearrange("p (t w) -> p t w", w=64)
                nc.scalar.dma_start(st_dst, st_src)

            if relax:
                relax_same_engine_deps(nc)

    return nc


def build_weights(kern: np.ndarray) -> np.ndarray:
    """4 banded lhsT matrices [K=128(in row), M=128(out row)], one per
    horizontal tap b: lhsT_b[k, m] = kern[m+1-k, b]; block-diag per image."""
    kern = np.asarray(kern, np.float32)
    w = np.zeros((128, 4 * 128), np.float32)
    for b in range(4):
        for blk in (0, 64):
            for m in range(64):
                for a in range(4):
                    k = m + 1 - a
                    if 0 <= k < 64:
                        w[blk + k, 128 * b + blk + m] = kern[a, b]
    return w


def marshal(x: np.ndarray, n_cores: int = N_CORES) -> np.ndarray:
    """Full (G, 64, 64) f32 -> prepadded per-core strips
    [n_cores, N_STRIPS, 128, STRIP_W]."""
    G = x.shape[0]
    n_strips = G // (n_cores * 2 * S)
    xr = x.reshape(n_cores, n_strips, S, 2, H, W)          # [c, s, t, j, r, w]
    out = np.zeros((n_cores, n_strips, 128, STRIP_W), np.float32)
    view = out[:, :, :, LEAD : LEAD + SLOT * S].reshape(
        n_cores, n_strips, 2, H, S, SLOT
    )                                                       # [c, s, j, r, t, u]
    view[..., 0:64] = xr.transpose(0, 1, 3, 4, 2, 5)
    return out


def make_in_maps(x: np.ndarray, kern: np.ndarray):
    """x: (B, C, 64, 64) f32 -> per-core input maps."""
    G = x.shape[0] * x.shape[1]
    xp = marshal(x.reshape(G, H, W))
    w_all = build_weights(kern)
    return [{"x": xp[c], "w": w_all} for c in range(N_CORES)]


_CACHE: dict = {}


def _get_nc():
    if "nc" not in _CACHE:
        _CACHE["nc"] = build_nc(n_strips=N_STRIPS)
    return _CACHE["nc"]


def kernel(x, kernel):
    x = np.ascontiguousarray(np.asarray(x, dtype=np.float32))
    kern = np.asarray(kernel, dtype=np.float32)
    B, C, HH, WW = x.shape

    nc = _get_nc()
    in_maps = make_in_maps(x, kern)
    res = run_bass_kernel_spmd(nc, in_maps, list(range(N_CORES)))
    y = np.concatenate([res.results[c]["y"] for c in range(N_CORES)], axis=0)
    return y.reshape(B, C, HH, WW).astype(np.float32)


if __name__ == "__main__":
    # quick self-check against numpy on random data (runs on hardware)
    rng = np.random.default_rng(0)
    x = rng.standard_normal((16, 512, 64, 64), dtype=np.float32)
    k1 = np.array([1.0, 3.0, 3.0, 1.0], np.float32)
    kern = np.outer(k1, k1)
    kern /= kern.sum()
    y = kernel(x, kern)
    print("out shape", y.shape, "dtype", y.dtype)


# revision 19
# speedup vs baseline: 1.3800x; 1.3800x over previous
"""Depthwise 4x4 FIR blur (upfirdn2d-style) on 8 Trainium2 NeuronCores.

Input  x: (16, 512, 64, 64) f32, kernel: (4, 4) f32 (normalized binomial).
Output y: same shape as x, y[g] = conv2d(zero-pad(x[g], (2,1)x(2,1)), flip(kernel)).

Equivalent per-image formula (derived from the reference):
    y[i, j] = sum_{a,b in [0,4)} kernel[a, b] * x[i+1-a, j+1-b]   (zero outside)

Strategy (per core, 1024 images = 16 strips of 64):
  - Host prepads each strip into [128, 2116]: partition k in [0,64) = row k of
    the even image of a pair, k in [64,128) = row k-64 of the odd image; along
    the free dim 32 image pairs at stride 66 (64 data cols + 2 zero cols) plus
    4 lead zeros. Horizontal taps then become free-dim shifts whose
    out-of-image reads land on zeros; strips load as one dense 1MB DMA.
  - The whole separable conv runs on the TensorEngine: for each horizontal tap
    b, a banded 128x128 matrix W_b (vertical taps folded in, block-diagonal
    per image) multiplies a shifted slice of the strip; 4 float32r matmuls
    accumulate per PSUM bank (1 cycle/row at N>=256).
  - ScalarE evacuates PSUM -> packed SBUF out tile and issues the stores
    (same engine -> the store needs no data semaphore), which go out in the
    natural [images, 64, 64] layout.
"""

import numpy as np

import concourse.bass as bass
import concourse.tile as tile
from concourse import mybir
from concourse.bass_utils import run_bass_kernel_spmd

# The kernel-tail drain waits on every semaphore family the kernel touched
# (PE + ACT + up to 8 DMA lanes); walrus rejects instructions with that many
# sync waits. Split the drain into several drain instructions, each carrying
# at most 3 waits — semantically identical (SP executes them in sequence).
import bass_rust as _bass_rust
from concourse.tile_scheduler import N_PROCS as _N_PROCS


def _split_drain_and_barrier(self, tick_clock, wait_clock):
    ScopedClock = _bass_rust.ScopedClock
    VectorClock = _bass_rust.VectorClock
    gc = tick_clock.global_clock
    vals = [gc[p] for p in range(_N_PROCS)]
    nonzero = [p for p in range(_N_PROCS) if vals[p] > 0]
    for p in nonzero:
        pv = [vals[q] if q == p else 0 for q in range(_N_PROCS)]
        d = self.nc.sync.drain()
        wait_clock.add_sem_waits(d.ins, ScopedClock({None: VectorClock(pv)}))
    self.nc.sync.drain()

    self.nc.all_engine_barrier()
    assert self.sems is not None
    popped = self.nc._tile_sem_poison_stack.pop()
    assert popped is self._sem_poison
    self.nc.clear_and_free_semaphores(list(self.sems.allocated().values()))
    self.nc.all_engine_barrier()


tile.TileContext._drain_and_barrier = _split_drain_and_barrier

# Partition HWDGE DMA-completion lanes by issuing engine: SP (loads) cycles
# lanes 0-5, ACT (stores) alternates lanes 6-7. A DMA must wait for the
# previous DMA on its lane (sem-value determinism); with dedicated store
# lanes that predecessor is store(s-2), whose completion the evacuation
# "poke" already made ACT observe — so the wait elides and every store keeps
# a single sem wait (walrus limit).
import concourse.tile_sem_assignment as _tsa
from concourse import bass_isa as _bass_isa


def _assign_tick_lane_split(self, inst):
    engine = inst.engine
    eng_proc_idx = (
        _tsa.ENGINE_SEQUENCER_TO_IDX if inst.is_sequencer_only() else _tsa.ENGINE_TO_IDX
    )[engine]
    if isinstance(inst, _tsa.DMAInst) and not isinstance(
        inst, _bass_isa.UserSyncedRemoteDMADescs
    ):
        if engine == mybir.EngineType.Pool:
            inst_proc_idx = _tsa.PROC_NAME_TO_IDX[f"DMASW{self.next_sw_dma_idx}"]
            self.next_sw_dma_idx = (self.next_sw_dma_idx + 1) % self.swdge_sem_count
        elif engine == mybir.EngineType.Activation:
            n = getattr(self, "_act_dma_count", 0)
            inst_proc_idx = _tsa.PROC_NAME_TO_IDX[f"DMAHW{6 + (n % 2)}"]
            self._act_dma_count = n + 1
        else:
            inst_proc_idx = _tsa.PROC_NAME_TO_IDX[f"DMAHW{self.next_hw_dma_idx}"]
            self.next_hw_dma_idx = (self.next_hw_dma_idx + 1) % 6
    elif isinstance(inst, mybir.InstCollectiveCompute):
        inst_proc_idx = _tsa.PROC_NAME_TO_IDX["Collectives"]
    else:
        inst_proc_idx = eng_proc_idx

    if not inst.is_executable():
        if not isinstance(inst, _tsa.BassTileCriticalSection):
            return
    if isinstance(inst, _bass_isa.InstPseudoReloadLibraryIndex):
        return

    if inst.descendants or isinstance(inst, _tsa._DMA_OR_COLLECTIVE_TYPES):
        inst.bass_scheduled_tick = self.global_clock.advance(inst_proc_idx)
        inst.bass_scheduled_proc = inst_proc_idx
        inst.bass_scheduled_scope = self.scope_name
        self._proc_insts[self.root_scope_name][inst_proc_idx].append(inst)
        if getattr(inst, "gen_mode", 0) == 1 and inst_proc_idx != eng_proc_idx:
            eng_tick = self.global_clock.advance(eng_proc_idx)
            self.tc.prep_eng_ticks[inst.name] = (eng_proc_idx, eng_tick)
            self._prep_eng_names[self.root_scope_name].append(inst.name)


_tsa.TileClockTick._assign_tick = _assign_tick_lane_split

N_CORES = 8
H = W = 64
SLOT = 66                       # free-dim stride per image (64 data + 2 zero)
LEAD = 4                        # leading zero cols in a strip
S = 32                          # image pairs (slots) per strip
STRIP_W = LEAD + SLOT * S       # 2116 f32 per partition
N_STRIPS = 16                   # strips per core (16 * 64 = 1024 images)
# chunk = slot range processed by one PSUM bank (<=512 f32 out cols)
CHUNKS = [(0, 7), (7, 14), (14, 21), (21, 28), (28, 32)]

F32 = mybir.dt.float32
F32R = mybir.dt.float32r


def _chunk_geom(t0, t1):
    ns = t1 - t0
    n_cols = SLOT * (ns - 1) + 64          # contiguous out span incl. gaps
    o = LEAD + SLOT * t0                   # first data col of the chunk
    return ns, n_cols, o


def build_nc(n_strips: int, mm_dtype=F32R, relax: bool = True):
    """Build the Bass program for one core processing n_strips*64 images.

    Sync-topology note: walrus allows only ONE semaphore wait on most
    instruction structs (matmul/ldweights, DMA pseudo), so the program is
    shaped so every instruction has at most one cross-engine dependency:
      - each strip gets its own SBUF x tile -> loads have NO deps at all
        (pure prefetch, all queued on the SP HWDGE ring up front);
      - tiny "absorber" matmuls fold the load-DMA wait and the PSUM-WAR
        (previous strip's DVE evacuations) into PE program order;
      - a 1-element DVE poke folds the out-buffer WAR (store of strip s-2)
        into DVE program order before the real evacuations.
    """
    from concourse.tile_rust import add_dep_helper as _adh
    from concourse.tile_scheduler import DMAInst

    def add_dep_helper(a, b, sync=False, reason=""):
        _adh(getattr(a, "ins", a), getattr(b, "ins", b), sync=sync, reason=reason)

    def relax_same_engine_deps(nc):
        """Demote same-engine compute->compute sync deps to order-only.

        Engines execute and complete their compute queues strictly in order,
        so a same-engine dependency never needs a semaphore — but Tile emits
        one anyway (self-waits), and walrus allows only a single sem wait on
        most instruction structs. DMA producers/consumers are excluded: a DMA
        instruction's completion is asynchronous to its issuing engine.
        """
        imap = nc.inst_map
        for inst in nc.all_instructions():
            if isinstance(inst, DMAInst) or not inst.is_executable():
                continue
            if inst.is_sequencer_only():
                continue
            sync_names = list(inst.sync_dependency_names())
            move = []
            for dn in sync_names:
                prod = imap.get(dn)
                if prod is None or isinstance(prod, DMAInst):
                    continue
                if not prod.is_executable() or prod.is_sequencer_only():
                    continue
                if prod.engine == inst.engine:
                    move.append(dn)
            if move:
                sync_set = inst.sync_dependency_set_copy()
                nosync_set = inst.nosync_dependency_set_copy()
                for dn in move:
                    sync_set.discard(dn)
                    nosync_set.add(dn)
                inst.set_sync_dependencies(sync_set)
                inst.set_nosync_dependencies(nosync_set)

    n_images = n_strips * 2 * S
    nc = bass.Bass(
        "TRN2", target_bir_lowering=False, detect_race_conditions=not relax
    )
    x_dram = nc.dram_tensor(
        "x", [n_strips, 128, STRIP_W], mm_dtype, kind="ExternalInput"
    )
    w_dram = nc.dram_tensor("w", [128, 512], mm_dtype, kind="ExternalInput")
    y_dram = nc.dram_tensor(
        "y", [n_strips, 128, 64 * S], F32, kind="ExternalOutput"
    )

    with tile.TileContext(nc) as tc:
        with (
            tc.tile_pool(name="pers", bufs=1) as pers,
            tc.tile_pool(name="psum", bufs=7, space="PSUM") as pp,
        ):
            wt = pers.tile([128, 512], mm_dtype, tag="wt")
            nc.sync.dma_start(wt[:], w_dram[:])

            x_tiles = [
                pers.tile([128, STRIP_W], mm_dtype, tag=f"xs{i}", name=f"xst{i}")
                for i in range(n_strips)
            ]
            y_bufs = [
                pers.tile([128, 64 * S], F32, tag=f"y{i}", name=f"ybuf{i}")
                for i in range(2)
            ]

            # prefetch every strip: no deps -> no waits, SP ring streams them
            for s in range(n_strips):
                nc.sync.dma_start(x_tiles[s][:], x_dram[s])

            # scratch PSUM tile for the absorber matmuls
            warm = pp.tile([128, 128], F32, name="warm", tag="warm", bufs=1)
            prev_mm = nc.tensor.matmul(
                warm[:], wt[:, 0:128], wt[:, 0:128], start=True, stop=True
            )

            for s in range(n_strips):
                xb = x_tiles[s]
                yb = y_bufs[s % 2]

                # absorber 1: load(s) completion -> PE program order
                d1 = nc.tensor.matmul(
                    warm[:, 0:4], wt[:, 0:128], xb[:, 0:4], start=True, stop=True
                )
                add_dep_helper(d1, prev_mm, sync=False, reason="strip order")

                # ---- 4 banded matmuls per chunk, accumulated in PSUM ----
                psum_tiles = [
                    pp.tile([128, 512], F32, name=f"ps{s}_{ci}", tag="ps")
                    for ci in range(len(CHUNKS))
                ]
                first_mms = []
                for b in range(4):
                    lhsT = wt[:, 128 * b : 128 * (b + 1)]
                    d = 1 - b                      # horizontal tap shift
                    for ci, (t0, t1) in enumerate(CHUNKS):
                        ns, n_cols, o = _chunk_geom(t0, t1)
                        rhs = xb[:, o + d : o + d + n_cols]
                        mm = nc.tensor.matmul(
                            psum_tiles[ci][:, 0:n_cols],
                            lhsT,
                            rhs,
                            start=(b == 0),
                            stop=(b == 3),
                        )
                        if b == 0:
                            add_dep_helper(mm, d1, sync=False, reason="gate")
                        prev_mm = mm

                # absorber 3: store(s-2) completion -> ACT program order.
                # Pokes one element of yb (chunk-0's copy rewrites it next).
                d3 = nc.scalar.copy(yb[0:1, 0:1], wt[0:1, 0:1].bitcast(F32))

                # ---- evacuate PSUM -> packed out tile (VectorE) ----
                copies = []
                for ci, (t0, t1) in enumerate(CHUNKS):
                    ns, n_cols, o = _chunk_geom(t0, t1)
                    src_c = psum_tiles[ci][:, 0 : SLOT * ns].rearrange(
                        "p (t u) -> p t u", u=SLOT
                    )[:, :, 0:64]
                    dst_c = yb[:, 64 * t0 : 64 * t1].rearrange(
                        "p (t w) -> p t w", w=64
                    )
                    cp = nc.scalar.copy(dst_c, src_c)
                    add_dep_helper(cp, d3, sync=False, reason="poke order")
                    copies.append(cp)

                # ---- store: dense permuted dump (host inverse-permutes) ----
                nc.scalar.dma_start(y_dram[s], yb[:])

            if relax:
                relax_same_engine_deps(nc)

    if relax:
        _strip_self_satisfied_waits(nc)

    return nc


def _strip_self_satisfied_waits(nc):
    """Post-scheduling: drop sem waits already guaranteed by the issuing
    engine's own instruction stream (e.g. PE waiting on the PE semaphore for
    a PSUM-slot WAW against its own earlier matmuls — the pool allocator
    emits these during scheduling, after the dep-relaxation pass ran).

    Safe because an engine's compute instructions complete in stream order,
    and only increments issued synchronously by THIS engine's earlier
    non-DMA instructions are counted (DMA completions are asynchronous and
    excluded). Walrus allows one sem wait per instruction, so these
    redundant self-waits are the difference between compiling and not.
    """
    from concourse.tile_scheduler import DMAInst

    cum: dict = {}
    for inst in nc.all_instructions():
        si = inst.sync_info
        if si is None:
            continue
        c = cum.setdefault(str(inst.engine), {})
        waits = list(si.on_wait)
        keep = [
            w
            for w in waits
            if not (
                w.sync_type == "semaphore"
                and w.wait_mode == "sem-ge-imm"
                and w.wait_reg is None
                and c.get(w.ant_name, 0) >= w.wait_value
            )
        ]
        if len(keep) != len(waits):
            si.on_wait = keep
        if not isinstance(inst, DMAInst):
            for u in si.on_update:
                if u.sync_type == "semaphore" and u.update_mode == "sem-inc":
                    c[u.ant_name] = c.get(u.ant_name, 0) + (u.update_value or 1)


def build_weights(kern: np.ndarray) -> np.ndarray:
    """4 banded lhsT matrices [K=128(in row), M=128(out row)], one per
    horizontal tap b: lhsT_b[k, m] = kern[m+1-k, b]; block-diag per image."""
    kern = np.asarray(kern, np.float32)
    w = np.zeros((128, 4 * 128), np.float32)
    for b in range(4):
        for blk in (0, 64):
            for m in range(64):
                for a in range(4):
                    k = m + 1 - a
                    if 0 <= k < 64:
                        w[blk + k, 128 * b + blk + m] = kern[a, b]
    return w


def marshal(x: np.ndarray, n_cores: int = N_CORES) -> np.ndarray:
    """Full (G, 64, 64) f32 -> prepadded per-core strips
    [n_cores, N_STRIPS, 128, STRIP_W]."""
    G = x.shape[0]
    n_strips = G // (n_cores * 2 * S)
    xr = x.reshape(n_cores, n_strips, S, 2, H, W)          # [c, s, t, j, r, w]
    out = np.zeros((n_cores, n_strips, 128, STRIP_W), np.float32)
    view = out[:, :, :, LEAD : LEAD + SLOT * S].reshape(
        n_cores, n_strips, 2, H, S, SLOT
    )                                                       # [c, s, j, r, t, u]
    view[..., 0:64] = xr.transpose(0, 1, 3, 4, 2, 5)
    return out


def unmarshal_y(yp: np.ndarray) -> np.ndarray:
    """Per-core permuted output [n_cores, N_STRIPS, 128, 64*S] -> (G, 64, 64)."""
    n_cores, n_strips = yp.shape[0], yp.shape[1]
    v = yp.reshape(n_cores, n_strips, 2, H, S, 64)         # [c, s, j, r, t, w]
    return np.ascontiguousarray(
        v.transpose(0, 1, 4, 2, 3, 5)                      # [c, s, t, j, r, w]
    ).reshape(n_cores * n_strips * 2 * S, H, W)


def make_in_maps(x: np.ndarray, kern: np.ndarray):
    """x: (B, C, 64, 64) f32 -> per-core input maps."""
    G = x.shape[0] * x.shape[1]
    xp = marshal(x.reshape(G, H, W))
    w_all = build_weights(kern)
    return [{"x": xp[c], "w": w_all} for c in range(N_CORES)]


_CACHE: dict = {}


def _get_nc():
    if "nc" not in _CACHE:
        _CACHE["nc"] = build_nc(n_strips=N_STRIPS)
    return _CACHE["nc"]


def kernel(x, kernel):
    x = np.ascontiguousarray(np.asarray(x, dtype=np.float32))
    kern = np.asarray(kernel, dtype=np.float32)
    B, C, HH, WW = x.shape

    nc = _get_nc()
    in_maps = make_in_maps(x, kern)
    res = run_bass_kernel_spmd(nc, in_maps, list(range(N_CORES)))
    yp = np.stack([res.results[c]["y"] for c in range(N_CORES)], axis=0)
    return unmarshal_y(yp).reshape(B, C, HH, WW).astype(np.float32)


if __name__ == "__main__":
    # quick self-check against numpy on random data (runs on hardware)
    rng = np.random.default_rng(0)
    x = rng.standard_normal((16, 512, 64, 64), dtype=np.float32)
    k1 = np.array([1.0, 3.0, 3.0, 1.0], np.float32)
    kern = np.outer(k1, k1)
    kern /= kern.sum()
    y = kernel(x, kern)
    print("out shape", y.shape, "dtype", y.dtype)


# revision 21
# speedup vs baseline: 1.3965x; 1.0120x over previous
"""Depthwise 4x4 FIR blur (upfirdn2d-style) on 8 Trainium2 NeuronCores.

Input  x: (16, 512, 64, 64) f32, kernel: (4, 4) f32 (normalized binomial).
Output y: same shape as x, y[g] = conv2d(zero-pad(x[g], (2,1)x(2,1)), flip(kernel)).

Equivalent per-image formula (derived from the reference):
    y[i, j] = sum_{a,b in [0,4)} kernel[a, b] * x[i+1-a, j+1-b]   (zero outside)

Strategy (per core, 1024 images = 16 strips of 64):
  - Host prepads each strip into [128, 2116]: partition k in [0,64) = row k of
    the even image of a pair, k in [64,128) = row k-64 of the odd image; along
    the free dim 32 image pairs at stride 66 (64 data cols + 2 zero cols) plus
    4 lead zeros. Horizontal taps then become free-dim shifts whose
    out-of-image reads land on zeros; strips load as one dense 1MB DMA.
  - The whole separable conv runs on the TensorEngine: for each horizontal tap
    b, a banded 128x128 matrix W_b (vertical taps folded in, block-diagonal
    per image) multiplies a shifted slice of the strip; 4 float32r matmuls
    accumulate per PSUM bank (1 cycle/row at N>=256).
  - ScalarE evacuates PSUM -> packed SBUF out tile and issues the stores
    (same engine -> the store needs no data semaphore), which go out in the
    natural [images, 64, 64] layout.
"""

import numpy as np

import concourse.bass as bass
import concourse.tile as tile
from concourse import mybir
from concourse.bass_utils import run_bass_kernel_spmd

# The kernel-tail drain waits on every semaphore family the kernel touched
# (PE + ACT + up to 8 DMA lanes); walrus rejects instructions with that many
# sync waits. Split the drain into several drain instructions, each carrying
# at most 3 waits — semantically identical (SP executes them in sequence).
import bass_rust as _bass_rust
from concourse.tile_scheduler import N_PROCS as _N_PROCS

# Enable walrus's LDWEIGHTS dedup pass (consecutive matmuls reusing the same
# stationary operand skip the reload). concourse disables it by default; it
# verified correct on hardware for this kernel.
import concourse.bass_utils as _bu

_orig_run_command = _bu.run_command


def _run_command_ldw_opt(argv, **kw):
    argv = [
        "--enable-ldw-opt=true" if a == "--enable-ldw-opt=false" else a for a in argv
    ]
    return _orig_run_command(argv, **kw)


_bu.run_command = _run_command_ldw_opt


def _split_drain_and_barrier(self, tick_clock, wait_clock):
    ScopedClock = _bass_rust.ScopedClock
    VectorClock = _bass_rust.VectorClock
    gc = tick_clock.global_clock
    vals = [gc[p] for p in range(_N_PROCS)]
    nonzero = [p for p in range(_N_PROCS) if vals[p] > 0]
    for p in nonzero:
        pv = [vals[q] if q == p else 0 for q in range(_N_PROCS)]
        d = self.nc.sync.drain()
        wait_clock.add_sem_waits(d.ins, ScopedClock({None: VectorClock(pv)}))
    self.nc.sync.drain()

    self.nc.all_engine_barrier()
    assert self.sems is not None
    popped = self.nc._tile_sem_poison_stack.pop()
    assert popped is self._sem_poison
    self.nc.clear_and_free_semaphores(list(self.sems.allocated().values()))
    self.nc.all_engine_barrier()


tile.TileContext._drain_and_barrier = _split_drain_and_barrier

# Partition HWDGE DMA-completion lanes by issuing engine: SP (loads) cycles
# lanes 0-5, ACT (stores) alternates lanes 6-7. A DMA must wait for the
# previous DMA on its lane (sem-value determinism); with dedicated store
# lanes that predecessor is store(s-2), whose completion the evacuation
# "poke" already made ACT observe — so the wait elides and every store keeps
# a single sem wait (walrus limit).
import concourse.tile_sem_assignment as _tsa
from concourse import bass_isa as _bass_isa


def _assign_tick_lane_split(self, inst):
    engine = inst.engine
    eng_proc_idx = (
        _tsa.ENGINE_SEQUENCER_TO_IDX if inst.is_sequencer_only() else _tsa.ENGINE_TO_IDX
    )[engine]
    if isinstance(inst, _tsa.DMAInst) and not isinstance(
        inst, _bass_isa.UserSyncedRemoteDMADescs
    ):
        if engine == mybir.EngineType.Pool:
            inst_proc_idx = _tsa.PROC_NAME_TO_IDX[f"DMASW{self.next_sw_dma_idx}"]
            self.next_sw_dma_idx = (self.next_sw_dma_idx + 1) % self.swdge_sem_count
        elif engine == mybir.EngineType.Activation:
            n = getattr(self, "_act_dma_count", 0)
            inst_proc_idx = _tsa.PROC_NAME_TO_IDX[f"DMAHW{6 + (n % 2)}"]
            self._act_dma_count = n + 1
        else:
            inst_proc_idx = _tsa.PROC_NAME_TO_IDX[f"DMAHW{self.next_hw_dma_idx}"]
            self.next_hw_dma_idx = (self.next_hw_dma_idx + 1) % 6
    elif isinstance(inst, mybir.InstCollectiveCompute):
        inst_proc_idx = _tsa.PROC_NAME_TO_IDX["Collectives"]
    else:
        inst_proc_idx = eng_proc_idx

    if not inst.is_executable():
        if not isinstance(inst, _tsa.BassTileCriticalSection):
            return
    if isinstance(inst, _bass_isa.InstPseudoReloadLibraryIndex):
        return

    if inst.descendants or isinstance(inst, _tsa._DMA_OR_COLLECTIVE_TYPES):
        inst.bass_scheduled_tick = self.global_clock.advance(inst_proc_idx)
        inst.bass_scheduled_proc = inst_proc_idx
        inst.bass_scheduled_scope = self.scope_name
        self._proc_insts[self.root_scope_name][inst_proc_idx].append(inst)
        if getattr(inst, "gen_mode", 0) == 1 and inst_proc_idx != eng_proc_idx:
            eng_tick = self.global_clock.advance(eng_proc_idx)
            self.tc.prep_eng_ticks[inst.name] = (eng_proc_idx, eng_tick)
            self._prep_eng_names[self.root_scope_name].append(inst.name)


_tsa.TileClockTick._assign_tick = _assign_tick_lane_split

N_CORES = 8
H = W = 64
SLOT = 66                       # free-dim stride per image (64 data + 2 zero)
LEAD = 4                        # leading zero cols in a strip
S = 32                          # image pairs (slots) per strip
STRIP_W = LEAD + SLOT * S       # 2116 f32 per partition
N_STRIPS = 16                   # strips per core (16 * 64 = 1024 images)
# chunk = slot range processed by one PSUM bank (<=512 f32 out cols)
CHUNKS = [(0, 7), (7, 14), (14, 21), (21, 28), (28, 32)]

F32 = mybir.dt.float32
F32R = mybir.dt.float32r


def _chunk_geom(t0, t1):
    ns = t1 - t0
    n_cols = SLOT * (ns - 1) + 64          # contiguous out span incl. gaps
    o = LEAD + SLOT * t0                   # first data col of the chunk
    return ns, n_cols, o


def build_nc(n_strips: int, mm_dtype=F32R, relax: bool = True, bmap=(0, 1, 1, 0)):
    """Build the Bass program for one core processing n_strips*64 images.

    Sync-topology note: walrus allows only ONE semaphore wait on most
    instruction structs (matmul/ldweights, DMA pseudo), so the program is
    shaped so every instruction has at most one cross-engine dependency:
      - each strip gets its own SBUF x tile -> loads have NO deps at all
        (pure prefetch, all queued on the SP HWDGE ring up front);
      - tiny "absorber" matmuls fold the load-DMA wait and the PSUM-WAR
        (previous strip's DVE evacuations) into PE program order;
      - a 1-element DVE poke folds the out-buffer WAR (store of strip s-2)
        into DVE program order before the real evacuations.
    """
    from concourse.tile_rust import add_dep_helper as _adh
    from concourse.tile_scheduler import DMAInst

    def add_dep_helper(a, b, sync=False, reason=""):
        _adh(getattr(a, "ins", a), getattr(b, "ins", b), sync=sync, reason=reason)

    def relax_same_engine_deps(nc):
        """Demote same-engine compute->compute sync deps to order-only.

        Engines execute and complete their compute queues strictly in order,
        so a same-engine dependency never needs a semaphore — but Tile emits
        one anyway (self-waits), and walrus allows only a single sem wait on
        most instruction structs. DMA producers/consumers are excluded: a DMA
        instruction's completion is asynchronous to its issuing engine.
        """
        imap = nc.inst_map
        for inst in nc.all_instructions():
            if isinstance(inst, DMAInst) or not inst.is_executable():
                continue
            if inst.is_sequencer_only():
                continue
            sync_names = list(inst.sync_dependency_names())
            move = []
            for dn in sync_names:
                prod = imap.get(dn)
                if prod is None or isinstance(prod, DMAInst):
                    continue
                if not prod.is_executable() or prod.is_sequencer_only():
                    continue
                if prod.engine == inst.engine:
                    move.append(dn)
            if move:
                sync_set = inst.sync_dependency_set_copy()
                nosync_set = inst.nosync_dependency_set_copy()
                for dn in move:
                    sync_set.discard(dn)
                    nosync_set.add(dn)
                inst.set_sync_dependencies(sync_set)
                inst.set_nosync_dependencies(nosync_set)

    n_images = n_strips * 2 * S
    nc = bass.Bass(
        "TRN2", target_bir_lowering=False, detect_race_conditions=not relax
    )
    x_dram = nc.dram_tensor(
        "x", [n_strips, 128, STRIP_W], mm_dtype, kind="ExternalInput"
    )
    w_dram = nc.dram_tensor("w", [128, 512], mm_dtype, kind="ExternalInput")
    y_dram = nc.dram_tensor(
        "y", [n_strips, 128, 64 * S], F32, kind="ExternalOutput"
    )

    with tile.TileContext(nc) as tc:
        with (
            tc.tile_pool(name="pers", bufs=1) as pers,
            tc.tile_pool(name="psum", bufs=7, space="PSUM") as pp,
        ):
            wt = pers.tile([128, 512], mm_dtype, tag="wt")
            nc.sync.dma_start(wt[:], w_dram[:])

            x_tiles = [
                pers.tile([128, STRIP_W], mm_dtype, tag=f"xs{i}", name=f"xst{i}")
                for i in range(n_strips)
            ]
            y_bufs = [
                pers.tile([128, 64 * S], F32, tag=f"y{i}", name=f"ybuf{i}")
                for i in range(2)
            ]

            # prefetch every strip: no deps -> no waits, SP ring streams them
            for s in range(n_strips):
                nc.sync.dma_start(x_tiles[s][:], x_dram[s])

            # scratch PSUM tile for the absorber matmuls
            warm = pp.tile([128, 128], F32, name="warm", tag="warm", bufs=1)
            prev_mm = nc.tensor.matmul(
                warm[:], wt[:, 0:128], wt[:, 0:128], start=True, stop=True
            )

            for s in range(n_strips):
                xb = x_tiles[s]
                yb = y_bufs[s % 2]

                # absorber 1: load(s) completion -> PE program order
                d1 = nc.tensor.matmul(
                    warm[:, 0:4], wt[:, 0:128], xb[:, 0:4], start=True, stop=True
                )
                add_dep_helper(d1, prev_mm, sync=False, reason="strip order")

                # ---- 4 banded matmuls per chunk, accumulated in PSUM ----
                psum_tiles = [
                    pp.tile([128, 512], F32, name=f"ps{s}_{ci}", tag="ps")
                    for ci in range(len(CHUNKS))
                ]
                first_mms = []
                for b in range(4):
                    u = bmap[b]
                    lhsT = wt[:, 128 * u : 128 * (u + 1)]
                    d = 1 - b                      # horizontal tap shift
                    for ci, (t0, t1) in enumerate(CHUNKS):
                        ns, n_cols, o = _chunk_geom(t0, t1)
                        rhs = xb[:, o + d : o + d + n_cols]
                        mm = nc.tensor.matmul(
                            psum_tiles[ci][:, 0:n_cols],
                            lhsT,
                            rhs,
                            start=(b == 0),
                            stop=(b == 3),
                        )
                        add_dep_helper(mm, prev_mm, sync=False, reason="pe chain")
                        prev_mm = mm

                # absorber 3: store(s-2) completion -> ACT program order.
                # Pokes one element of yb (chunk-0's copy rewrites it next).
                d3 = nc.scalar.copy(yb[0:1, 0:1], wt[0:1, 0:1].bitcast(F32))

                # ---- evacuate PSUM -> packed out tile (VectorE) ----
                copies = []
                for ci, (t0, t1) in enumerate(CHUNKS):
                    ns, n_cols, o = _chunk_geom(t0, t1)
                    src_c = psum_tiles[ci][:, 0 : SLOT * ns].rearrange(
                        "p (t u) -> p t u", u=SLOT
                    )[:, :, 0:64]
                    dst_c = yb[:, 64 * t0 : 64 * t1].rearrange(
                        "p (t w) -> p t w", w=64
                    )
                    cp = nc.scalar.copy(dst_c, src_c)
                    add_dep_helper(cp, d3, sync=False, reason="poke order")
                    copies.append(cp)

                # ---- store: dense permuted dump (host inverse-permutes) ----
                nc.scalar.dma_start(y_dram[s], yb[:])

            if relax:
                relax_same_engine_deps(nc)

    if relax:
        _strip_self_satisfied_waits(nc)

    return nc


def _strip_self_satisfied_waits(nc):
    """Post-scheduling: drop sem waits already guaranteed by the issuing
    engine's own instruction stream (e.g. PE waiting on the PE semaphore for
    a PSUM-slot WAW against its own earlier matmuls — the pool allocator
    emits these during scheduling, after the dep-relaxation pass ran).

    Safe because an engine's compute instructions complete in stream order,
    and only increments issued synchronously by THIS engine's earlier
    non-DMA instructions are counted (DMA completions are asynchronous and
    excluded). Walrus allows one sem wait per instruction, so these
    redundant self-waits are the difference between compiling and not.
    """
    from concourse.tile_scheduler import DMAInst

    cum: dict = {}
    for inst in nc.all_instructions():
        si = inst.sync_info
        if si is None:
            continue
        c = cum.setdefault(str(inst.engine), {})
        waits = list(si.on_wait)
        keep = [
            w
            for w in waits
            if not (
                w.sync_type == "semaphore"
                and w.wait_mode == "sem-ge-imm"
                and w.wait_reg is None
                and c.get(w.ant_name, 0) >= w.wait_value
            )
        ]
        if len(keep) != len(waits):
            si.on_wait = keep
        if not isinstance(inst, DMAInst):
            for u in si.on_update:
                if u.sync_type == "semaphore" and u.update_mode == "sem-inc":
                    c[u.ant_name] = c.get(u.ant_name, 0) + (u.update_value or 1)


def tap_map(kern: np.ndarray):
    """Map each horizontal tap b to a unique kernel column index (the
    binomial kernel has col0==col3 and col1==col2, letting consecutive
    matmuls share one stationary operand so walrus elides the reload)."""
    kern = np.asarray(kern, np.float32)
    uniq, bmap = [], []
    for b in range(4):
        for ui, u in enumerate(uniq):
            if np.array_equal(kern[:, b], u):
                bmap.append(ui)
                break
        else:
            bmap.append(len(uniq))
            uniq.append(kern[:, b])
    return uniq, bmap


def build_weights(kern: np.ndarray) -> np.ndarray:
    """Banded lhsT matrices [K=128(in row), M=128(out row)], one per unique
    kernel column u: lhsT_u[k, m] = kern[m+1-k, col]; block-diag per image.
    Always padded to 4 slots of 128 so the DRAM tensor shape is static."""
    kern = np.asarray(kern, np.float32)
    uniq, _ = tap_map(kern)
    w = np.zeros((128, 4 * 128), np.float32)
    for ui, col in enumerate(uniq):
        for blk in (0, 64):
            for m in range(64):
                for a in range(4):
                    k = m + 1 - a
                    if 0 <= k < 64:
                        w[blk + k, 128 * ui + blk + m] = col[a]
    return w


def marshal(x: np.ndarray, n_cores: int = N_CORES) -> np.ndarray:
    """Full (G, 64, 64) f32 -> prepadded per-core strips
    [n_cores, N_STRIPS, 128, STRIP_W]."""
    G = x.shape[0]
    n_strips = G // (n_cores * 2 * S)
    xr = x.reshape(n_cores, n_strips, S, 2, H, W)          # [c, s, t, j, r, w]
    out = np.zeros((n_cores, n_strips, 128, STRIP_W), np.float32)
    view = out[:, :, :, LEAD : LEAD + SLOT * S].reshape(
        n_cores, n_strips, 2, H, S, SLOT
    )                                                       # [c, s, j, r, t, u]
    view[..., 0:64] = xr.transpose(0, 1, 3, 4, 2, 5)
    return out


def unmarshal_y(yp: np.ndarray) -> np.ndarray:
    """Per-core permuted output [n_cores, N_STRIPS, 128, 64*S] -> (G, 64, 64)."""
    n_cores, n_strips = yp.shape[0], yp.shape[1]
    v = yp.reshape(n_cores, n_strips, 2, H, S, 64)         # [c, s, j, r, t, w]
    return np.ascontiguousarray(
        v.transpose(0, 1, 4, 2, 3, 5)                      # [c, s, t, j, r, w]
    ).reshape(n_cores * n_strips * 2 * S, H, W)


def make_in_maps(x: np.ndarray, kern: np.ndarray):
    """x: (B, C, 64, 64) f32 -> per-core input maps."""
    G = x.shape[0] * x.shape[1]
    xp = marshal(x.reshape(G, H, W))
    w_all = build_weights(kern)
    return [{"x": xp[c], "w": w_all} for c in range(N_CORES)]


_CACHE: dict = {}


def _get_nc(bmap=(0, 1, 1, 0)):
    key = ("nc", tuple(bmap))
    if key not in _CACHE:
        _CACHE[key] = build_nc(n_strips=N_STRIPS, bmap=tuple(bmap))
    return _CACHE[key]


def kernel(x, kernel):
    x = np.ascontiguousarray(np.asarray(x, dtype=np.float32))
    kern = np.asarray(kernel, dtype=np.float32)
    B, C, HH, WW = x.shape

    _, bmap = tap_map(kern)
    nc = _get_nc(bmap)
    in_maps = make_in_maps(x, kern)
    res = run_bass_kernel_spmd(nc, in_maps, list(range(N_CORES)))
    yp = np.stack([res.results[c]["y"] for c in range(N_CORES)], axis=0)
    return unmarshal_y(yp).reshape(B, C, HH, WW).astype(np.float32)


if __name__ == "__main__":
    # quick self-check against numpy on random data (runs on hardware)
    rng = np.random.default_rng(0)
    x = rng.standard_normal((16, 512, 64, 64), dtype=np.float32)
    k1 = np.array([1.0, 3.0, 3.0, 1.0], np.float32)
    kern = np.outer(k1, k1)
    kern /= kern.sum()
    y = kernel(x, kern)
    print("out shape", y.shape, "dtype", y.dtype)
